# revision 2
# baseline (speedup 1.0000x reference)
"""LDS (diagonal linear state space + AR) kernel for 8 Trainium2 cores, v1.1.

Computation (per batch b):
    uB[t, s]   = sum_d x[t, d] * B[d, s]
    h[t]       = A * h[t-1] + uB[t]          (h[-1] = h0, A diagonal)
    lds[t, o]  = sum_s h[t, s] * C[s, o]
    out[t, o]  = sum_{i<10} sum_d M[o, d, i] * x[t-i, d]  +  lds[t+10, o]

Sharding: data-parallel over batch, 2 batches per core, no collectives.

Precision strategy (validated numerically on host):
  - Output is dominated by the AR term; the LDS term is ~100x smaller.
    LDS path (uB matmul, scan, C matmul) in fp8e4 with DoubleRow matmuls
    (K=256/pass); AR matmuls in bf16. End-to-end rel err ~8e-3 on HW
    (threshold 2e-2); the scan state is fp32 internally.

Schedule (v1.1): the AR matmuls depend only on x, so they are emitted in
the same chunk as the uB matmuls; the C matmuls for chunk j are emitted
during chunk j+1 (after the scans of chunk j land). This keeps the PE
busy during the scan-paced fill of each batch. Out tiles are packed in
pairs into full PSUM banks [128, 2x256]. Weight tensors are loaded with
one DMA each and DMA issue is spread across the sync and scalar queues.
"""

import sys

if "/opt/trn_rl_repo" not in sys.path:
    sys.path.insert(0, "/opt/trn_rl_repo")

import numpy as np

import concourse.bass as bass
import concourse.mybir as mybir
from concourse.tile import TileContext

BSZ = 16
SEQ = 2048
D = 256  # input dim
S = 1024  # state dim
O = 256  # output dim
KX = 10
N_CORES = 8
B_PER_CORE = BSZ // N_CORES  # 2

PAD = 16  # left zero-pad on xb time for the AR taps (needs >= KX-1 = 9)
HPAD = 16  # right zero-pad on h time for the +10 shift (needs >= KX)
TCH = 512  # uB psum / scan chunk width (= 1 PSUM bank of fp32)
OTCH = 128  # out tile time width (= partition dim of out psum tile)
NCH = SEQ // TCH  # 4 chunks per batch

F32 = mybir.dt.float32
BF16 = mybir.dt.bfloat16
FP8 = mybir.dt.float8e4
DR = mybir.MatmulPerfMode.DoubleRow

_CACHED = {}


def _build_nc():
    nc = bass.Bass()

    x8_d = nc.dram_tensor("x8", [B_PER_CORE, 128, 2, SEQ], FP8,
                          kind="ExternalInput")
    xb_d = nc.dram_tensor("xb", [B_PER_CORE, 2, 128, PAD + SEQ], BF16,
                          kind="ExternalInput")
    b8_d = nc.dram_tensor("b8", [128, 2, 8 * 128], FP8, kind="ExternalInput")
    c8_d = nc.dram_tensor("c8", [128, 2, 4 * O], FP8, kind="ExternalInput")
    m_d = nc.dram_tensor("mmat", [128, KX * 2 * O], BF16,
                         kind="ExternalInput")
    ah_d = nc.dram_tensor("ah", [128, 16], F32, kind="ExternalInput")
    out_d = nc.dram_tensor("out", [B_PER_CORE, SEQ, O], BF16,
                           kind="ExternalOutput")

    with TileContext(nc) as tc:
        with tc.tile_pool(name="persist", bufs=1) as persist, \
             tc.tile_pool(name="h8p", bufs=8) as h8_pool, \
             tc.tile_pool(name="outsb", bufs=4) as out_sbuf, \
             tc.tile_pool(name="ubps", bufs=4, space="PSUM") as ub_psum, \
             tc.tile_pool(name="outps", bufs=4, space="PSUM") as out_psum:

            # ---- persistent operands ----
            # Only 3 DMA queues exist (sync, scalar, gpsimd-SWDGE), each
            # ~126 GB/s. Interleave the loads so chunk-0 compute (uB, then
            # AR) can start as early as possible.
            ah = persist.tile([128, 16], F32, tag="ah")
            b8 = persist.tile([128, 2, 8 * 128], FP8, tag="b8")
            mt = persist.tile([128, KX * 2 * O], BF16, tag="mt")
            c8 = persist.tile([128, 2, 4 * O], FP8, tag="c8")
            x8 = {}
            for b in range(B_PER_CORE):
                x8[b] = persist.tile([128, 2, SEQ], FP8, tag=f"x8{b}",
                                     name=f"x8{b}")
            xb = {}
            for b in range(B_PER_CORE):
                for dch in range(2):
                    t = persist.tile([128, PAD + SEQ], BF16, tag=f"xb{b}{dch}")
                    xb[b, dch] = t

            def xb_dma(eng, b, dch, tch):
                c0 = 0 if tch == 0 else PAD + tch * TCH
                c1 = PAD + (tch + 1) * TCH
                eng.dma_start(out=xb[b, dch][:, c0:c1],
                              in_=xb_d[b, dch][:, c0:c1])

            # sync queue: uB-critical, then batch-1 x8
            nc.sync.dma_start(out=b8[:], in_=b8_d[:])
            nc.sync.dma_start(out=x8[0][:, :, 0:TCH], in_=x8_d[0][:, :, 0:TCH])
            nc.sync.dma_start(out=ah[:], in_=ah_d[:])
            for tch in range(1, NCH):
                t0 = tch * TCH
                nc.sync.dma_start(out=x8[0][:, :, t0:t0 + TCH],
                                  in_=x8_d[0][:, :, t0:t0 + TCH])
            for tch in range(NCH):
                t0 = tch * TCH
                nc.sync.dma_start(out=x8[1][:, :, t0:t0 + TCH],
                                  in_=x8_d[1][:, :, t0:t0 + TCH])
            # scalar queue: AR-critical (M taps 0-4, xb chunk 0), then rest
            MH = KX * O  # half of the M columns (taps 0-4)
            nc.scalar.dma_start(out=mt[:, 0:MH], in_=m_d[:, 0:MH])
            xb_dma(nc.scalar, 0, 0, 0)
            xb_dma(nc.scalar, 0, 1, 0)
            nc.scalar.dma_start(out=mt[:, MH:], in_=m_d[:, MH:])
            xb_dma(nc.scalar, 0, 0, 1)
            xb_dma(nc.scalar, 0, 1, 1)
            nc.scalar.dma_start(out=c8[:], in_=c8_d[:])
            for tch in range(2, NCH):
                xb_dma(nc.scalar, 0, 0, tch)
                xb_dma(nc.scalar, 0, 1, tch)
            for dch in range(2):
                nc.scalar.dma_start(out=xb[1, dch][:], in_=xb_d[1, dch])

            def mm_w(i, dch):
                c0 = (i * 2 + dch) * O
                return mt[:, c0:c0 + O]

            def emit_ar_pair(b, t0p):
                """AR matmuls for out tiles (t0p, t0p+128) into one psum
                bank [128, 2*256]; each half is its own accumulation
                group (start on first, stop on last AR matmul)."""
                ops = out_psum.tile([128, 2 * O], F32, name="ops")
                # start=True zeroes PSUM at 2KB-bank granularity, so only
                # the very first matmul into the bank may carry it; the
                # second half accumulates onto the pending-zero region.
                n = 0
                for half in range(2):
                    t0o = t0p + half * OTCH
                    for i in range(KX):
                        for dch in range(2):
                            nc.tensor.matmul(
                                out=ops[:, half * O:half * O + O],
                                lhsT=xb[b, dch][:, PAD + t0o - i:
                                                PAD + t0o - i + OTCH],
                                rhs=mm_w(i, dch),
                                start=(n == 0),
                                stop=(n == 2 * KX - 1),
                                skip_group_check=True,
                            )
                            n += 1
                return ops

            def emit_c_pair(b, h8, t0p, ops):
                """C matmuls (fp8 DR, accumulate post-stop) + store."""
                for half in range(2):
                    t0o = t0p + half * OTCH
                    for k in range(4):
                        nc.tensor.matmul(
                            out=ops[:, half * O:half * O + O],
                            lhsT=h8[k][:, :, t0o + KX:t0o + KX + OTCH],
                            rhs=c8[:, :, k * O:k * O + O],
                            start=False,
                            stop=False,
                            perf_mode=DR,
                            skip_group_check=True,
                        )
                osb = out_sbuf.tile([128, 2 * O], BF16)
                nc.scalar.copy(out=osb[:], in_=ops[:])
                out_eng = nc.scalar if b == 0 else nc.sync
                for half in range(2):
                    t0o = t0p + half * OTCH
                    out_eng.dma_start(
                        out=out_d[b, t0o:t0o + OTCH, :],
                        in_=osb[:, half * O:half * O + O])

            # ---- per-batch pipeline ----
            for b in range(B_PER_CORE):
                h8 = []
                for k in range(4):
                    t = h8_pool.tile([128, 2, SEQ + HPAD], FP8, tag="h8")
                    # zero tail for the +KX shift (read by C matmuls)
                    nc.gpsimd.memset(t[:, :, SEQ:], 0.0)
                    h8.append(t)

                ar_done = {}
                for tch in range(NCH):
                    t0 = tch * TCH
                    for wave in range(2):
                        for sch in range(wave * 4, wave * 4 + 4):
                            ub = ub_psum.tile([128, TCH], F32)
                            for half in range(2):
                                u0 = t0 + half * 256
                                nc.tensor.matmul(
                                    out=ub[:, half * 256:half * 256 + 256],
                                    lhsT=b8[:, :, sch * 128:sch * 128 + 128],
                                    rhs=x8[b][:, :, u0:u0 + 256],
                                    start=True,
                                    stop=True,
                                    perf_mode=DR,
                                )
                            k, j = divmod(sch, 2)
                            init = (ah[:, 8 + sch:9 + sch] if tch == 0
                                    else h8[k][:, j, t0 - 1:t0])
                            nc.vector.tensor_tensor_scan(
                                out=h8[k][:, j, t0:t0 + TCH],
                                data0=ah[:, sch:sch + 1].broadcast_to(
                                    [128, TCH]),
                                data1=ub[:],
                                initial=init,
                                op0=mybir.AluOpType.mult,
                                op1=mybir.AluOpType.add,
                            )
                        # AR for THIS chunk (depends only on x), C for the
                        # PREVIOUS chunk (its scans have landed)
                        p = wave * 2 * OTCH  # pair offset within chunk
                        ar_done[tch, wave] = emit_ar_pair(b, t0 + p)
                        if tch > 0:
                            emit_c_pair(b, h8, (tch - 1) * TCH + p,
                                        ar_done[tch - 1, wave])
                for wave in range(2):
                    emit_c_pair(b, h8, (NCH - 1) * TCH + wave * 2 * OTCH,
                                ar_done[NCH - 1, wave])

    import bass_rust as _br
    _br.move_matmul_waits_to_ldweights(nc.m)
    _br.generate_event_semaphores(nc)

    return nc


def _prep_core_inputs(inputs, h0, A, B, C, M, core):
    """Host-side shard + layout + quantization prep for one core."""
    import ml_dtypes
    f8 = ml_dtypes.float8_e4m3
    bf = ml_dtypes.bfloat16

    bs = slice(core * B_PER_CORE, (core + 1) * B_PER_CORE)
    x = np.asarray(inputs[bs], np.float32)  # [2, T, D]
    xtr = np.ascontiguousarray(x.transpose(0, 2, 1))  # [2, D, T]

    # x8[b, p, j, t] = x[b, t, j*128 + p]
    x8 = np.ascontiguousarray(
        xtr.reshape(B_PER_CORE, 2, 128, SEQ).transpose(0, 2, 1, 3)).astype(f8)

    xb = np.zeros((B_PER_CORE, 2, 128, PAD + SEQ), bf)
    xb[:, :, :, PAD:] = xtr.reshape(B_PER_CORE, 2, 128, SEQ).astype(bf)

    # b8[p, j, sch*128 + s_in] = B[j*128 + p, sch*128 + s_in]
    b8 = np.ascontiguousarray(
        B.reshape(2, 128, S).transpose(1, 0, 2)).astype(f8)
    # c8[p, j, k*256 + o] = C[(2k + j)*128 + p, o]
    c8 = np.ascontiguousarray(
        C.reshape(4, 2, 128, O).transpose(2, 1, 0, 3).reshape(
            128, 2, 4 * O)).astype(f8)
    # mmat[p, (i*2 + dch)*256 + o] = M[o, dch*128 + p, i]
    mmat = np.ascontiguousarray(
        M.transpose(2, 1, 0).reshape(KX, 2, 128, O).transpose(2, 0, 1, 3)
        .reshape(128, KX * 2 * O)).astype(bf)
    ah = np.zeros((128, 16), np.float32)
    ah[:, :8] = A.reshape(8, 128).T
    ah[:, 8:] = h0.reshape(8, 128).T
    return {"x8": x8, "xb": xb, "b8": b8, "c8": c8, "mmat": mmat, "ah": ah}


LAST_RESULT = None


def kernel(inputs, h0, A, B, C, M):
    global LAST_RESULT
    from concourse.bass_utils import run_bass_kernel_spmd

    inputs = np.asarray(inputs, np.float32)
    h0 = np.asarray(h0, np.float32)
    A = np.asarray(A, np.float32)
    B = np.asarray(B, np.float32)
    C = np.asarray(C, np.float32)
    M = np.asarray(M, np.float32)

    if "nc" not in _CACHED:
        _CACHED["nc"] = _build_nc()
    nc = _CACHED["nc"]

    in_maps = [_prep_core_inputs(inputs, h0, A, B, C, M, c)
               for c in range(N_CORES)]
    res = run_bass_kernel_spmd(nc, in_maps, list(range(N_CORES)))
    LAST_RESULT = res
    out = np.concatenate([np.asarray(res.results[c]["out"], np.float32)
                          for c in range(N_CORES)], axis=0)
    return out


# revision 3
# speedup vs baseline: 1.0053x; 1.0053x over previous
"""LDS (diagonal linear state space + AR) kernel for 8 Trainium2 cores, v1.1.

Computation (per batch b):
    uB[t, s]   = sum_d x[t, d] * B[d, s]
    h[t]       = A * h[t-1] + uB[t]          (h[-1] = h0, A diagonal)
    lds[t, o]  = sum_s h[t, s] * C[s, o]
    out[t, o]  = sum_{i<10} sum_d M[o, d, i] * x[t-i, d]  +  lds[t+10, o]

Sharding: data-parallel over batch, 2 batches per core, no collectives.

Precision strategy (validated numerically on host):
  - Output is dominated by the AR term; the LDS term is ~100x smaller.
    LDS path (uB matmul, scan, C matmul) in fp8e4 with DoubleRow matmuls
    (K=256/pass); AR matmuls in bf16. End-to-end rel err ~8e-3 on HW
    (threshold 2e-2); the scan state is fp32 internally.

Schedule (v1.1): the AR matmuls depend only on x, so they are emitted in
the same chunk as the uB matmuls; the C matmuls for chunk j are emitted
during chunk j+1 (after the scans of chunk j land). This keeps the PE
busy during the scan-paced fill of each batch. Out tiles are packed in
pairs into full PSUM banks [128, 2x256]. Weight tensors are loaded with
one DMA each and DMA issue is spread across the sync and scalar queues.
"""

import sys

if "/opt/trn_rl_repo" not in sys.path:
    sys.path.insert(0, "/opt/trn_rl_repo")

import numpy as np

import concourse.bass as bass
import concourse.mybir as mybir
from concourse.tile import TileContext

BSZ = 16
SEQ = 2048
D = 256  # input dim
S = 1024  # state dim
O = 256  # output dim
KX = 10
N_CORES = 8
B_PER_CORE = BSZ // N_CORES  # 2

PAD = 16  # left zero-pad on xb time for the AR taps (needs >= KX-1 = 9)
HPAD = 16  # right zero-pad on h time for the +10 shift (needs >= KX)
TCH = 512  # uB psum / scan chunk width (= 1 PSUM bank of fp32)
OTCH = 128  # out tile time width (= partition dim of out psum tile)
NCH = SEQ // TCH  # 4 chunks per batch

F32 = mybir.dt.float32
BF16 = mybir.dt.bfloat16
FP8 = mybir.dt.float8e4
DR = mybir.MatmulPerfMode.DoubleRow

_CACHED = {}


def _build_nc():
    nc = bass.Bass()

    x8_d = nc.dram_tensor("x8", [B_PER_CORE, 128, 2, SEQ], FP8,
                          kind="ExternalInput")
    xb_d = nc.dram_tensor("xb", [B_PER_CORE, 2, 128, PAD + SEQ], BF16,
                          kind="ExternalInput")
    b8_d = nc.dram_tensor("b8", [128, 2, 8 * 128], FP8, kind="ExternalInput")
    c8_d = nc.dram_tensor("c8", [128, 2, 4 * O], FP8, kind="ExternalInput")
    m_d = nc.dram_tensor("mmat", [128, KX * 2 * O], BF16,
                         kind="ExternalInput")
    ah_d = nc.dram_tensor("ah", [128, 16], F32, kind="ExternalInput")
    out_d = nc.dram_tensor("out", [B_PER_CORE, SEQ, O], BF16,
                           kind="ExternalOutput")

    with TileContext(nc) as tc:
        with tc.tile_pool(name="persist", bufs=1) as persist, \
             tc.tile_pool(name="h8p", bufs=8) as h8_pool, \
             tc.tile_pool(name="outsb", bufs=4) as out_sbuf, \
             tc.tile_pool(name="ubps", bufs=4, space="PSUM") as ub_psum, \
             tc.tile_pool(name="outps", bufs=4, space="PSUM") as out_psum:

            # ---- persistent operands ----
            # Only 3 DMA queues exist (sync, scalar, gpsimd-SWDGE), each
            # ~126 GB/s. Interleave the loads so chunk-0 compute (uB, then
            # AR) can start as early as possible.
            ah = persist.tile([128, 16], F32, tag="ah")
            b8 = persist.tile([128, 2, 8 * 128], FP8, tag="b8")
            mt = persist.tile([128, KX * 2 * O], BF16, tag="mt")
            c8 = persist.tile([128, 2, 4 * O], FP8, tag="c8")
            x8 = {}
            for b in range(B_PER_CORE):
                x8[b] = persist.tile([128, 2, SEQ], FP8, tag=f"x8{b}",
                                     name=f"x8{b}")
            xb = {}
            for b in range(B_PER_CORE):
                for dch in range(2):
                    t = persist.tile([128, PAD + SEQ], BF16, tag=f"xb{b}{dch}")
                    xb[b, dch] = t

            def xb_dma(eng, b, dch, tch):
                c0 = 0 if tch == 0 else PAD + tch * TCH
                c1 = PAD + (tch + 1) * TCH
                eng.dma_start(out=xb[b, dch][:, c0:c1],
                              in_=xb_d[b, dch][:, c0:c1])

            # sync queue: uB-critical in fine grains so the very first
            # matmuls gate on ~100KB, not on whole tensors
            nc.sync.dma_start(out=b8[:, :, 0:512], in_=b8_d[:, :, 0:512])
            nc.sync.dma_start(out=x8[0][:, :, 0:256], in_=x8_d[0][:, :, 0:256])
            nc.sync.dma_start(out=ah[:], in_=ah_d[:])
            nc.sync.dma_start(out=x8[0][:, :, 256:TCH],
                              in_=x8_d[0][:, :, 256:TCH])
            nc.sync.dma_start(out=b8[:, :, 512:], in_=b8_d[:, :, 512:])
            for tch in range(1, NCH):
                t0 = tch * TCH
                nc.sync.dma_start(out=x8[0][:, :, t0:t0 + TCH],
                                  in_=x8_d[0][:, :, t0:t0 + TCH])
            for tch in range(NCH):
                t0 = tch * TCH
                nc.sync.dma_start(out=x8[1][:, :, t0:t0 + TCH],
                                  in_=x8_d[1][:, :, t0:t0 + TCH])
            # scalar queue: AR-critical (M tap-by-tap for the first taps,
            # xb chunk 0), then the rest
            nc.scalar.dma_start(out=mt[:, 0:512], in_=m_d[:, 0:512])
            xb_dma(nc.scalar, 0, 0, 0)
            xb_dma(nc.scalar, 0, 1, 0)
            nc.scalar.dma_start(out=mt[:, 512:2048], in_=m_d[:, 512:2048])
            nc.scalar.dma_start(out=mt[:, 2048:], in_=m_d[:, 2048:])
            xb_dma(nc.scalar, 0, 0, 1)
            xb_dma(nc.scalar, 0, 1, 1)
            nc.scalar.dma_start(out=c8[:], in_=c8_d[:])
            for tch in range(2, NCH):
                xb_dma(nc.scalar, 0, 0, tch)
                xb_dma(nc.scalar, 0, 1, tch)
            for dch in range(2):
                nc.scalar.dma_start(out=xb[1, dch][:], in_=xb_d[1, dch])

            def mm_w(i, dch):
                c0 = (i * 2 + dch) * O
                return mt[:, c0:c0 + O]

            def emit_ar_pair(b, t0p):
                """AR matmuls for out tiles (t0p, t0p+128) into one psum
                bank [128, 2*256]; each half is its own accumulation
                group (start on first, stop on last AR matmul)."""
                ops = out_psum.tile([128, 2 * O], F32, name="ops")
                # start=True zeroes PSUM at 2KB-bank granularity, so only
                # the very first matmul into the bank may carry it; the
                # second half accumulates onto the pending-zero region.
                n = 0
                for half in range(2):
                    t0o = t0p + half * OTCH
                    for i in range(KX):
                        for dch in range(2):
                            nc.tensor.matmul(
                                out=ops[:, half * O:half * O + O],
                                lhsT=xb[b, dch][:, PAD + t0o - i:
                                                PAD + t0o - i + OTCH],
                                rhs=mm_w(i, dch),
                                start=(n == 0),
                                stop=(n == 2 * KX - 1),
                                skip_group_check=True,
                            )
                            n += 1
                return ops

            def emit_c_pair(b, h8, t0p, ops):
                """C matmuls (fp8 DR, accumulate post-stop) + store."""
                for half in range(2):
                    t0o = t0p + half * OTCH
                    for k in range(4):
                        nc.tensor.matmul(
                            out=ops[:, half * O:half * O + O],
                            lhsT=h8[k][:, :, t0o + KX:t0o + KX + OTCH],
                            rhs=c8[:, :, k * O:k * O + O],
                            start=False,
                            stop=False,
                            perf_mode=DR,
                            skip_group_check=True,
                        )
                osb = out_sbuf.tile([128, 2 * O], BF16)
                nc.scalar.copy(out=osb[:], in_=ops[:])
                out_eng = nc.scalar if b == 0 else nc.sync
                # one DMA for both 128-row tiles: sbuf [p, h*256+o] maps to
                # dram rows t0p + h*128 + p
                out_eng.dma_start(
                    out=out_d[b, t0p:t0p + 2 * OTCH, :].rearrange(
                        "(h p) o -> p h o", h=2),
                    in_=osb[:])

            # ---- per-batch pipeline ----
            for b in range(B_PER_CORE):
                h8 = []
                for k in range(4):
                    t = h8_pool.tile([128, 2, SEQ + HPAD], FP8, tag="h8")
                    # zero tail for the +KX shift (read by C matmuls)
                    nc.gpsimd.memset(t[:, :, SEQ:], 0.0)
                    h8.append(t)

                ar_done = {}
                for tch in range(NCH):
                    t0 = tch * TCH
                    for wave in range(2):
                        for sch in range(wave * 4, wave * 4 + 4):
                            ub = ub_psum.tile([128, TCH], F32)
                            for half in range(2):
                                u0 = t0 + half * 256
                                nc.tensor.matmul(
                                    out=ub[:, half * 256:half * 256 + 256],
                                    lhsT=b8[:, :, sch * 128:sch * 128 + 128],
                                    rhs=x8[b][:, :, u0:u0 + 256],
                                    start=True,
                                    stop=True,
                                    perf_mode=DR,
                                )
                            k, j = divmod(sch, 2)
                            init = (ah[:, 8 + sch:9 + sch] if tch == 0
                                    else h8[k][:, j, t0 - 1:t0])
                            nc.vector.tensor_tensor_scan(
                                out=h8[k][:, j, t0:t0 + TCH],
                                data0=ah[:, sch:sch + 1].broadcast_to(
                                    [128, TCH]),
                                data1=ub[:],
                                initial=init,
                                op0=mybir.AluOpType.mult,
                                op1=mybir.AluOpType.add,
                            )
                        # AR for THIS chunk (depends only on x), C for the
                        # PREVIOUS chunk (its scans have landed)
                        p = wave * 2 * OTCH  # pair offset within chunk
                        ar_done[tch, wave] = emit_ar_pair(b, t0 + p)
                        if tch > 0:
                            emit_c_pair(b, h8, (tch - 1) * TCH + p,
                                        ar_done[tch - 1, wave])
                for wave in range(2):
                    emit_c_pair(b, h8, (NCH - 1) * TCH + wave * 2 * OTCH,
                                ar_done[NCH - 1, wave])

    import bass_rust as _br
    _br.move_matmul_waits_to_ldweights(nc.m)
    _br.generate_event_semaphores(nc)

    return nc


def _prep_core_inputs(inputs, h0, A, B, C, M, core):
    """Host-side shard + layout + quantization prep for one core."""
    import ml_dtypes
    f8 = ml_dtypes.float8_e4m3
    bf = ml_dtypes.bfloat16

    bs = slice(core * B_PER_CORE, (core + 1) * B_PER_CORE)
    x = np.asarray(inputs[bs], np.float32)  # [2, T, D]
    xtr = np.ascontiguousarray(x.transpose(0, 2, 1))  # [2, D, T]

    # x8[b, p, j, t] = x[b, t, j*128 + p]
    x8 = np.ascontiguousarray(
        xtr.reshape(B_PER_CORE, 2, 128, SEQ).transpose(0, 2, 1, 3)).astype(f8)

    xb = np.zeros((B_PER_CORE, 2, 128, PAD + SEQ), bf)
    xb[:, :, :, PAD:] = xtr.reshape(B_PER_CORE, 2, 128, SEQ).astype(bf)

    # b8[p, j, sch*128 + s_in] = B[j*128 + p, sch*128 + s_in]
    b8 = np.ascontiguousarray(
        B.reshape(2, 128, S).transpose(1, 0, 2)).astype(f8)
    # c8[p, j, k*256 + o] = C[(2k + j)*128 + p, o]
    c8 = np.ascontiguousarray(
        C.reshape(4, 2, 128, O).transpose(2, 1, 0, 3).reshape(
            128, 2, 4 * O)).astype(f8)
    # mmat[p, (i*2 + dch)*256 + o] = M[o, dch*128 + p, i]
    mmat = np.ascontiguousarray(
        M.transpose(2, 1, 0).reshape(KX, 2, 128, O).transpose(2, 0, 1, 3)
        .reshape(128, KX * 2 * O)).astype(bf)
    ah = np.zeros((128, 16), np.float32)
    ah[:, :8] = A.reshape(8, 128).T
    ah[:, 8:] = h0.reshape(8, 128).T
    return {"x8": x8, "xb": xb, "b8": b8, "c8": c8, "mmat": mmat, "ah": ah}


LAST_RESULT = None


def kernel(inputs, h0, A, B, C, M):
    global LAST_RESULT
    from concourse.bass_utils import run_bass_kernel_spmd

    inputs = np.asarray(inputs, np.float32)
    h0 = np.asarray(h0, np.float32)
    A = np.asarray(A, np.float32)
    B = np.asarray(B, np.float32)
    C = np.asarray(C, np.float32)
    M = np.asarray(M, np.float32)

    if "nc" not in _CACHED:
        _CACHED["nc"] = _build_nc()
    nc = _CACHED["nc"]

    in_maps = [_prep_core_inputs(inputs, h0, A, B, C, M, c)
               for c in range(N_CORES)]
    res = run_bass_kernel_spmd(nc, in_maps, list(range(N_CORES)))
    LAST_RESULT = res
    out = np.concatenate([np.asarray(res.results[c]["out"], np.float32)
                          for c in range(N_CORES)], axis=0)
    return out
